# revision 21
# baseline (speedup 1.0000x reference)
"""DCT sequence-compression kernel for TRN2 (nn_CompressedModel).

For x [B=64, T=1024, D=768] fp32 computes (matching the reference):
  x_dct = (C_T @ x)[:, :k, :]          k = 922
  x_rec = C_k^T @ x_dct
returning (x_rec, x_dct).

Structure exploited (all folds are host-side data prep / host-side
recombination; the device only runs dense matmuls):

  DCT-II mirror symmetry on the input index, applied twice:
    e  = x[:512] + rev(x[512:]),   o  = x[:512] - rev(x[512:])
    ee = e[:256] + rev(e[256:]),   eo = e[:256] - rev(e[256:])
    dct[4j]   = Wee^T ee   (Wee = C_T[0:922:4, :256]^T, [256, 231])
    dct[4j+2] = Weo^T eo   (Weo = C_T[2:922:4, :256]^T, [256, 230])
    dct[2j+1] = Wo^T  o    (Wo  = C_T[1:922:2, :512]^T, [512, 461])

  DCT-III mirror symmetry on the output index, applied twice:
    rs[n] = As^T dct_ee    (As = C_k[0:922:4, :231], [231, 231])
    ra[n] = Aa^T dct_eo    (Aa = C_k[2:922:4, :231], [230, 231])
    ro[n] = Ao^T dct_o     (Ao = C_k[1:922:2, :461], [461, 461])
    re[n] = rs[n] + ra[n],  re[460-n] = rs[n] - ra[n]
    rec[n] = re[n] + ro[n], rec[921-n] = re[n] - ro[n]

vs the naive dual matmul this is ~2.4x less tensor-engine streaming.

Implementation notes (probe-driven):
  * All matmul operands bf16: PE streams 1 elem/cycle regardless of
    dtype, so bf16 is free on the PE; it halves HBM traffic and (with
    128-column weight tiles) enables FWL fast weight loads.
  * Everything is padded to uniform 128-row chunks with zero weight
    columns/rows, so every LDWEIGHTS is a full 128-column FWL load and
    PSUM/copy/DMA tiles are uniform. Zero padding keeps the math exact.
  * Per-dma_start fixed cost on the HWDGE rings dominated the previous
    version (+90us for 19 DMAs/batch): inputs are packed host-side into
    one [128, 8, 768] tensor (1 DMA/batch) and outputs into four
    [128, 4, 768] partition-major padded tensors (4 DMAs/batch).
  * PSUM accumulates fp32; PSUM->SBUF copies downcast to bf16, split
    across VectorE and ScalarE so neither gates the PE.
  * Host upcasts/combines the bf16 outputs (rel err ~4e-3, gate 2e-2).
Pure data parallel over B across 8 cores.
"""

import os

import numpy as np
import ml_dtypes

# The trimmed axon environment has no NTFF profile hook; make sure
# run_bass_kernel_spmd never tries the trace path.
os.environ["BASS_NEVER_TRACE"] = "1"

import concourse.bass as bass  # noqa: F401
import concourse.mybir as mybir
import concourse.tile as tile
from concourse import bacc
from concourse.bass_utils import run_bass_kernel_spmd

B, T, D = 64, 1024, 768
K = 922              # ceil(0.9 * 1024)
H = T // 2           # 512: o contraction length
Q = T // 4           # 256: ee/eo contraction length
NE = 461             # odd dct rows / rec mirror half
NQ = 231             # rows k%4==0  (also rs/ra output rows)
NQ2 = 230            # rows k%4==2
N_CORES = 8
BPC = B // N_CORES   # batches per core
P = 128
N0 = 512             # first free-dim split (PSUM bank width in fp32)

BF16 = mybir.dt.bfloat16
NPBF16 = ml_dtypes.bfloat16


def _dct_matrix(N: int) -> np.ndarray:
    """Orthonormal DCT-II matrix [N, N] in float64."""
    n = np.arange(N, dtype=np.float64)
    C = np.cos(np.pi * (2.0 * n[None, :] + 1.0) * n[:, None] / (2.0 * N))
    s = np.full(N, np.sqrt(2.0 / N))
    s[0] = np.sqrt(1.0 / N)
    return s[:, None] * C


def _pack_w(W: np.ndarray, ncc: int, nout_pad: int) -> np.ndarray:
    """[rows, nout] -> [128, ncc, nout_pad] partition-major bf16,
    zero-padding rows to ncc*128 and columns to nout_pad."""
    rows, nout = W.shape
    Wp = np.zeros((ncc * P, nout_pad))
    Wp[:rows, :nout] = W
    return np.ascontiguousarray(
        Wp.reshape(ncc, P, nout_pad).transpose(1, 0, 2).astype(NPBF16))


def _build_weights():
    C_T = _dct_matrix(T)
    C_k = _dct_matrix(K)
    return {
        "wee": _pack_w(C_T[0:K:4, 0:Q].T, 2, 2 * P),   # [128, 2, 256]
        "weo": _pack_w(C_T[2:K:4, 0:Q].T, 2, 2 * P),
        "wo": _pack_w(C_T[1:K:2, 0:H].T, 4, 4 * P),    # [128, 4, 512]
        "as_": _pack_w(C_k[0:K:4, 0:NQ], 2, 2 * P),
        "aa": _pack_w(C_k[2:K:4, 0:NQ], 2, 2 * P),
        "ao": _pack_w(C_k[1:K:2, 0:NE], 4, 4 * P),
    }


W_SHAPES = [("wee", 2), ("weo", 2), ("wo", 4), ("as_", 2), ("aa", 2),
            ("ao", 4)]

# (weight key, n contraction chunks, n output chunks)
MAT_EE = ("wee", 2, 2)
MAT_EO = ("weo", 2, 2)
MAT_O = ("wo", 4, 4)
MAT_RS = ("as_", 2, 2)
MAT_RA = ("aa", 2, 2)
MAT_RO = ("ao", 4, 4)


def _build_bass(loop_repeat: int = 1, probe: str = "full"):
    """loop_repeat>1 wraps the program in a hardware For_i loop (same
    outputs each trip) — used by test.py for slope-based HW timing.
    probe: knockout mode for overhead attribution ("full", "noout" =
    skip output DMAs, "nocopy" = also skip PSUM copies, "noin" = also
    hoist input DMAs out of the loop). Non-full modes produce garbage
    outputs and are for timing only."""
    no_out = probe in ("noout", "nocopy", "noin")
    no_copy = probe in ("nocopy", "noin")
    static_in = probe == "noin"
    tiny = probe == "tinycopy"
    # free-split knockouts: emit only the N=512 (or only the N=256) MM
    splits = {"n512": [(0, N0)], "n256": [(N0, D)]}.get(
        probe, [(0, N0), (N0, D)])
    f32 = mybir.dt.float32
    nc = bacc.Bacc("TRN2", target_bir_lowering=False, debug=False,
                   num_devices=N_CORES)
    # packed input: chunk groups [ee(2), eo(2), o(4)]
    x_in = nc.dram_tensor("xin", [BPC, P, 8, D], BF16,
                          kind="ExternalInput").ap()
    w_in = {
        name: nc.dram_tensor(name, [P, ncc, ncc * P], BF16,
                             kind="ExternalInput").ap()
        for name, ncc in W_SHAPES
    }
    # packed padded outputs: da = [dee(2), deo(2)], db = [do(4)],
    # rt = [rs(2), ra(2)], rb = [ro(4)]
    da_out = nc.dram_tensor("da", [BPC, P, 4, D], BF16,
                            kind="ExternalOutput").ap()
    db_out = nc.dram_tensor("db", [BPC, P, 4, D], BF16,
                            kind="ExternalOutput").ap()
    rt_out = nc.dram_tensor("rt", [BPC, P, 4, D], BF16,
                            kind="ExternalOutput").ap()
    rb_out = nc.dram_tensor("rb", [BPC, P, 4, D], BF16,
                            kind="ExternalOutput").ap()

    with tile.TileContext(nc) as tc:
        with (
            tc.tile_pool(name="wp", bufs=1) as wp,
            tc.tile_pool(name="xp", bufs=3) as xp,
            tc.tile_pool(name="sp", bufs=3) as sp,
            tc.tile_pool(name="pp", bufs=2, space="PSUM") as pp,
        ):
            wt = {}
            for name, ncc in W_SHAPES:
                w_tile = wp.tile([P, ncc, ncc * P], BF16, tag=name)
                wt[name] = w_tile
                nc.scalar.dma_start(w_tile[:], w_in[name])

            def mm_mat(mat, rhs_pair, rhs_c0, dst_pair, c0, copy_eng):
                """One matrix for a PAIR of batches: per (out chunk,
                contraction chunk) the weight tile is loaded once and
                streamed by 4 MMs (2 batches x 2 free splits, N=512
                MMs first) — weight-changes halve vs per-batch issue.
                Copies go to bf16 SBUF column group (c0 + ci) of each
                batch's dst."""
                wkey, n_cc, n_out = mat
                wtile = wt[wkey]
                for ci in range(n_out):
                    r0 = ci * P
                    pt0_ = pp.tile([P, D], f32, tag="pt0")
                    pt1_ = pp.tile([P, D], f32, tag="pt1")
                    pts = [pt0_, pt1_]
                    for cc in range(n_cc):
                        st, sp_ = (cc == 0), (cc == n_cc - 1)
                        rc = rhs_c0 + cc
                        for (f0, f1) in splits:
                            for pt, rhs_tile in zip(pts, rhs_pair):
                                nc.tensor.matmul(
                                    pt[:, f0:f1],
                                    wtile[:, cc, r0:r0 + P],
                                    rhs_tile[:, rc, f0:f1],
                                    start=st, stop=sp_)
                    if no_copy:
                        return
                    for pt, dst in zip(pts, dst_pair):
                        if copy_eng == "v":
                            nc.vector.tensor_copy(dst[:, c0 + ci, :], pt[:])
                        else:
                            nc.scalar.copy(dst[:, c0 + ci, :], pt[:])

            if static_in:
                xt0 = wp.tile([P, 8, D], BF16, tag="xt0")
                nc.scalar.dma_start(xt0[:], x_in[0])
            if no_copy:
                da0 = wp.tile([P, 4, D], BF16, tag="da0")
                db0 = wp.tile([P, 4, D], BF16, tag="db0")
                rt0 = wp.tile([P, 4, D], BF16, tag="rt0")
                rb0 = wp.tile([P, 4, D], BF16, tag="rb0")
                for t_ in (da0, db0, rt0, rb0):
                    nc.scalar.dma_start(t_[:], x_in[0, :, 0:4, :])

            def body():
                for b in range(0, BPC, 2):
                    bp = (b, b + 1)
                    if static_in:
                        xts = (xt0, xt0)
                    else:
                        xts = []
                        for j in bp:
                            xt = xp.tile([P, 8, D], BF16, tag="xt")
                            nc.scalar.dma_start(xt[:], x_in[j])
                            xts.append(xt)
                    if no_copy:
                        das = (da0, da0)
                        dbs = (db0, db0)
                        rts = (rt0, rt0)
                        rbs = (rb0, rb0)
                    else:
                        das, dbs, rts, rbs = [], [], [], []
                        for j in bp:
                            da = sp.tile([P, 4, D], BF16, tag="da")
                            db = sp.tile([P, 4, D], BF16, tag="db")
                            rt = sp.tile([P, 4, D], BF16, tag="rt")
                            rb = sp.tile([P, 4, D], BF16, tag="rb")
                            das.append(da)
                            dbs.append(db)
                            rts.append(rt)
                            rbs.append(rb)
                    # dct parts (staged in SBUF as rec inputs)
                    mm_mat(MAT_EE, xts, 0, das, 0, "v")
                    mm_mat(MAT_EO, xts, 2, das, 2, "v")
                    if not no_out:
                        for j, da in zip(bp, das):
                            nc.sync.dma_start(da_out[j], da[:])
                    mm_mat(MAT_O, xts, 4, dbs, 0, "s")
                    if not no_out:
                        for j, db in zip(bp, dbs):
                            nc.sync.dma_start(db_out[j], db[:])
                    # rec parts (contract the staged dct parts)
                    mm_mat(MAT_RS, das, 0, rts, 0, "v")
                    mm_mat(MAT_RA, das, 2, rts, 2, "v")
                    if not no_out:
                        for j, rt in zip(bp, rts):
                            nc.sync.dma_start(rt_out[j], rt[:])
                    mm_mat(MAT_RO, dbs, 0, rbs, 0, "s")
                    if not no_out:
                        for j, rb in zip(bp, rbs):
                            nc.sync.dma_start(rb_out[j], rb[:])

            if loop_repeat > 1:
                with tc.For_i(0, loop_repeat, 1):
                    body()
            else:
                body()
            if no_out:
                # bind output tensors once (garbage contents)
                for ap in (da_out, db_out, rt_out, rb_out):
                    for b in range(BPC):
                        nc.sync.dma_start(ap[b], x_in[b, :, 0:4, :])
    nc.compile()
    return nc


_CACHE = {}


def _get():
    if "nc" not in _CACHE:
        _CACHE["nc"] = _build_bass()
        _CACHE["w"] = _build_weights()
    return _CACHE["nc"], _CACHE["w"]


def _make_in_maps(x: np.ndarray):
    _, w = _get()
    x = np.ascontiguousarray(x, dtype=np.float32)
    lo = x[:, :H, :]
    hi = x[:, :H - 1:-1, :]
    e = lo + hi
    o = lo - hi                          # [B, 512, D]
    ee = e[:, :Q] + e[:, :Q - 1:-1]      # [B, 256, D]
    eo = e[:, :Q] - e[:, :Q - 1:-1]
    # pack chunk groups [ee(2), eo(2), o(4)] -> [B, 128, 8, D] bf16
    packed = np.concatenate([
        ee.reshape(B, 2, P, D), eo.reshape(B, 2, P, D),
        o.reshape(B, 4, P, D)], axis=1)
    xin = np.ascontiguousarray(
        packed.transpose(0, 2, 1, 3).astype(NPBF16))
    sl = [slice(c * BPC, (c + 1) * BPC) for c in range(N_CORES)]
    return [{"xin": xin[s], **w} for s in sl]


def kernel(x: np.ndarray, _results_out=None):
    """x [64, 1024, 768] fp32 -> (x_rec [64, 922, 768], x_dct [64, 922, 768])."""
    nc, _ = _get()
    in_maps = _make_in_maps(x)
    res = run_bass_kernel_spmd(nc, in_maps, core_ids=list(range(N_CORES)))
    if _results_out is not None:
        _results_out.append(res)
    f32 = np.float32

    def cat(name):
        # [B, P, 4, D] -> [B, 512, D] (chunk-major rows), f32
        a = np.concatenate([r[name] for r in res.results], axis=0)
        return a.transpose(0, 2, 1, 3).reshape(B, 4 * P, D).astype(f32)

    da, db = cat("da"), cat("db")
    rt, rb = cat("rt"), cat("rb")
    dee, deo, do = da[:, 0:NQ], da[:, 2 * P:2 * P + NQ2], db[:, 0:NE]
    rs, ra, ro = rt[:, 0:NQ], rt[:, 2 * P:2 * P + NQ], rb[:, 0:NE]
    x_dct = np.empty((B, K, D), f32)
    x_dct[:, 0::4] = dee
    x_dct[:, 2::4] = deo
    x_dct[:, 1::2] = do
    re = np.empty((B, NE, D), f32)
    re[:, :NQ] = rs + ra
    re[:, NQ:] = (rs[:, :NQ2] - ra[:, :NQ2])[:, ::-1]
    x_rec = np.empty((B, K, D), f32)
    x_rec[:, :NE] = re + ro
    x_rec[:, NE:] = (re - ro)[:, ::-1]
    return x_rec, x_dct
